# revision 31
# baseline (speedup 1.0000x reference)
"""Trainium2 Bass kernel for Mistral-style MHA prefill (sparse_attention).

Problem: B=2, S=2048, DIM=4096, 32 q heads / 8 kv heads, head_dim=128,
sliding window 2048 (== S, so the mask is pure causal), RoPE, fp32 reference.

Sharding (8 cores): data-parallel over batch (2) x tensor-parallel over heads
(4).  Core c = b*4 + tp handles batch b, q-heads [tp*8, tp*8+8), kv-heads
[tp*2, tp*2+2).  wq/wk/wv are sharded column-wise (output dim), wo row-wise
(input dim); the all-reduce after wo is done on the host (numpy sum of the 4
fp32 partials per batch).

Design (v8, ~748us vs 877us v1 baseline; PE-bound, ~94% tensor-engine busy):
  - Transpose-free attention, q-block outer (flash-style): scores are computed
    TRANSPOSED, st[k, q-block] = K^T-block.T @ Q^T-block, for BOTH heads of a
    pair in one matmul (they share a kv head; the moving operand is a strided
    [dh, 2, 128] view of Q^T).  After exp on the scalar engine, st_exp is
    directly the lhsT of the PV matmul A[q, dh] += st_exp.T @ V_ext[kb] -- no
    128x128 PE transposes of P (v1 spent ~139k PE cycles + 17.8M PSUM-copy
    elements on those).
  - V is stored with a ones-column ([k, 129]); the PV matmul emits the softmax
    row-sum in column 128 for free (no activation-accumulator reads).  PSUM
    accumulation groups are bank-granular, so each head's A accumulator owns a
    full psum bank; only two are open at a time (q-block outer loop).
  - Projection/attention emission is interleaved (weighted round-robin
    generators) so dense projection matmuls cover the PE while exp runs;
    RoPE's pair-swap perm matmul is deferred 8 ko into the following
    projection group to hide the PSUM->SBUF copy latency.
  - Startup: weight DMAs are queued before the 8MB x^T half (DMA issue is
    serialized on the sync engine, ~1.4us+ per descriptor, 8-sem rotation);
    wq weights are double-buffered and prefetched one pair ahead; wo weights
    stream into x^T's dead buffer during the last attention pair.
  - fp8 DoubleRow was measured at only 2x bf16 FLOPs (224ns for a [128,512]
    K=256 matmul = same as bf16 K=128), so residual-compensated fp8 (the only
    scheme that passes the 2e-2 error gate; plain fp8 sims at 5e-2) would be
    1.5x SLOWER than bf16.  bf16 throughout.
"""

import os
import sys

import numpy as np

for _p in ("/opt/trn_rl_repo",):
    if _p not in sys.path and os.path.isdir(_p):
        sys.path.insert(0, _p)

import ml_dtypes  # noqa: E402

import concourse.bass as bass  # noqa: E402
import concourse.mybir as mybir  # noqa: E402
import concourse.tile as tile  # noqa: E402
from concourse.bass_utils import run_bass_kernel_spmd  # noqa: E402

BF16 = ml_dtypes.bfloat16


def _install_drain_split_patch():
    """The pinned walrus rejects Tile's kernel-tail Drain when it carries more
    than ~2 semaphore waits ("Too many sync wait commands").  Split the global
    drain's waits across trailing sync-engine nops (1 wait each); all waits
    still complete before the all-engine barrier and semaphore reset."""
    if getattr(tile.TileContext, "_drain_split_patched", False):
        return
    from concourse.vector_clock import ScopedClock

    limit = 1

    def _patched_dab(self, tick_clock, wait_clock):
        drain_inst = self.nc.sync.drain()
        raw = drain_inst.ins
        wait_clock.add_sem_waits(raw, ScopedClock({None: tick_clock.global_clock}))
        si = raw.sync_info
        waits = list(si.on_wait or [])
        if len(waits) > limit:
            si.on_wait = waits[:limit]
            for i in range(limit, len(waits), limit):
                nraw = self.nc.sync.nop().ins
                nsi = nraw.sync_info
                if nsi is None:
                    nraw.sync_info = mybir.SyncInfo(
                        on_wait=waits[i : i + limit], on_update=[]
                    )
                else:
                    nsi.on_wait = list(nsi.on_wait or []) + waits[i : i + limit]
        self.nc.all_engine_barrier()
        popped = self.nc._tile_sem_poison_stack.pop()
        assert popped is self._sem_poison
        self.nc.clear_and_free_semaphores(list(self.sems.allocated().values()))
        self.nc.all_engine_barrier()

    tile.TileContext._drain_and_barrier = _patched_dab
    tile.TileContext._drain_split_patched = True


_install_drain_split_patch()

P = 128
S = 2048
D = 4096
KO = D // P  # 32 contraction chunks
SH = S // 2  # half of the sequence
NH_L = 8  # q heads per core
NKV_L = 2  # kv heads per core
DH = 128
VW = 130  # v_t row stride (128 data + 1 ones + 1 pad)
SCALE = float(DH) ** -0.5
N_CORES = 8

_dt_f32 = mybir.dt.float32
_dt_bf16 = mybir.dt.bfloat16


def _rr(*gens):
    """Weighted round-robin over (generator, weight) pairs: pull `weight`
    steps from each live generator per rotation."""
    live = [[iter(g), w] for g, w in gens]
    while live:
        for ent in list(live):
            g, w = ent
            for _ in range(w):
                try:
                    next(g)
                except StopIteration:
                    live.remove(ent)
                    break


def _run(g):
    for _ in g:
        pass


def _emit(tc, aps):
    nc = tc.nc
    xr = aps["xT"].rearrange("(ko p) s -> p ko s", p=P)  # [128, 32, 2048]
    wqr = aps["wqT"].rearrange("(ko p) o -> p ko o", p=P)  # [128, 32, 1024]
    wkvr = aps["wkvT"].rearrange("(ko p) o -> p ko o", p=P)  # [128, 32, 512]
    wor = aps["woT"].rearrange("(ho p) e -> p ho e", p=P)  # [128, 8, 4096]
    out_ap = aps["out"]  # [2048, 4096] f32

    from contextlib import ExitStack

    with ExitStack() as g:
        singles = g.enter_context(tc.tile_pool(name="singles", bufs=1))
        small = g.enter_context(tc.tile_pool(name="small", bufs=8))
        a_pool = g.enter_context(tc.tile_pool(name="a_pool", bufs=1))
        kv_pool = g.enter_context(tc.tile_pool(name="kvp", bufs=1))
        ostage = g.enter_context(tc.tile_pool(name="ostage", bufs=3))
        ps_mm = g.enter_context(tc.tile_pool(name="ps_mm", bufs=2, space="PSUM"))
        ps_scr = g.enter_context(tc.tile_pool(name="ps_scr", bufs=4, space="PSUM"))
        xt_pool = g.enter_context(tc.tile_pool(name="xt", bufs=1))
        c1 = ExitStack()
        kvw_pool = c1.enter_context(tc.tile_pool(name="kvw", bufs=2))
        wq_pool = c1.enter_context(tc.tile_pool(name="wqp", bufs=2))
        qt_pool = c1.enter_context(tc.tile_pool(name="qtp", bufs=2))
        rope_pool = c1.enter_context(tc.tile_pool(name="rope", bufs=2))
        pb_pool = c1.enter_context(tc.tile_pool(name="pbp", bufs=4))
        ps_a = c1.enter_context(tc.tile_pool(name="ps_a", bufs=2, space="PSUM"))

        cexp_t = singles.tile([P, S], _dt_bf16)
        sexp_t = singles.tile([P, S], _dt_bf16)
        perm_t = singles.tile([P, P], _dt_bf16)
        ident_t = singles.tile([P, P], _dt_bf16)
        maskT_t = singles.tile([P, P], _dt_f32)


        # A[s, hd] for the whole core, bf16: [128, 16 s-blocks, 1024]
        a_t = a_pool.tile([P, 16, NH_L * DH], _dt_bf16)
        # K^T (roped) for the full sequence; V natural with a ones column
        kt_t = kv_pool.tile([P, NKV_L, S], _dt_bf16)
        v_t = kv_pool.tile([P, 16, NKV_L, VW], _dt_bf16)


        cp_flip = [0]

        def cp(out, in_):
            # Alternate PSUM->SBUF copies between the scalar and vector engines.
            if cp_flip[0] % 2 == 0:
                nc.scalar.copy(out=out, in_=in_)
            else:
                nc.vector.tensor_copy(out=out, in_=in_)
            cp_flip[0] += 1

        def rope_finish(dst, s_off, w):
            sw = ps_scr.tile([P, 512], _dt_f32, tag="scr", name="sw")
            nc.tensor.matmul(sw[:, :w], lhsT=perm_t, rhs=dst, start=True, stop=True)
            t1 = rope_pool.tile([P, 512], _dt_bf16, tag="t1")
            nc.vector.tensor_mul(t1[:, :w], sw[:, :w], sexp_t[:, s_off : s_off + w])
            nc.vector.tensor_mul(dst, dst, cexp_t[:, s_off : s_off + w])
            nc.vector.tensor_add(dst, dst, t1[:, :w])

        def rope_chunk(dst, psrc, s_off, w):
            """dst (sbuf bf16 [128, w]) <- rope(psrc (psum f32 [128, w]))."""
            nc.scalar.copy(out=dst, in_=psrc)
            rope_finish(dst, s_off, w)

        def dma_xt(xt, s0, eng):
            # first-needed-first: the opening V group reads s[0:128] across all
            # ko, so deliver that in quarter-ko strips before anything else.
            for s128 in range(2):
                for kq in range(4):
                    eng.dma_start(
                        out=xt[:, kq * 8 : (kq + 1) * 8, s128 * 128 : (s128 + 1) * 128],
                        in_=xr[
                            :,
                            kq * 8 : (kq + 1) * 8,
                            s0 + s128 * 128 : s0 + (s128 + 1) * 128,
                        ],
                    )
            for sblk in range(1, 4):
                for kq in range(4):
                    eng.dma_start(
                        out=xt[:, kq * 8 : (kq + 1) * 8, sblk * 256 : (sblk + 1) * 256],
                        in_=xr[
                            :,
                            kq * 8 : (kq + 1) * 8,
                            s0 + sblk * 256 : s0 + (sblk + 1) * 256,
                        ],
                    )

        def dma_wkv_v():
            # first ko-octet first: V's opening matmuls only need wkv[:, 0:8]
            wkv_v = kvw_pool.tile([P, KO, 256], _dt_bf16, tag="wkv", name="wkv_v")
            nc.sync.dma_start(out=wkv_v[:, 0:8, :], in_=wkvr[:, 0:8, 256:512])
            nc.sync.dma_start(out=wkv_v[:, 8:KO, :], in_=wkvr[:, 8:KO, 256:512])
            return wkv_v

        def gen_v_proj(xt, hi, wkv_v):
            """V projection (natural layout [s, dh+ones]) for one half."""
            for sbl in range(8):
                sb = hi * 8 + sbl
                gv = ps_mm.tile([P, 512], _dt_f32, tag="mm", name="gv")
                for ko in range(KO):
                    nc.tensor.matmul(
                        gv[:, :256],
                        lhsT=xt[:, ko, sbl * P : (sbl + 1) * P],
                        rhs=wkv_v[:, ko, :],
                        start=(ko == 0),
                        stop=(ko == KO - 1),
                    )
                cp(
                    v_t[:, sb, :, 0:DH],
                    gv[:, :256].rearrange("p (g d) -> p g d", g=2),
                )
                yield

        def k_proj(xt, hi):
            s0 = hi * SH
            wkv_k = kvw_pool.tile([P, KO, 256], _dt_bf16, tag="wkv", name="wkv_k")
            for i in range(2):
                nc.sync.dma_start(
                    out=wkv_k[:, i * 16 : (i + 1) * 16, :],
                    in_=wkvr[:, i * 16 : (i + 1) * 16, 0:256],
                )
            pend = [None]
            for g2 in range(NKV_L):
                for sc in range(2):
                    gk = ps_mm.tile([P, 512], _dt_f32, tag="mm", name="gk")
                    for ko in range(KO):
                        nc.tensor.matmul(
                            gk,
                            lhsT=wkv_k[:, ko, g2 * P : (g2 + 1) * P],
                            rhs=xt[:, ko, sc * 512 : (sc + 1) * 512],
                            start=(ko == 0),
                            stop=(ko == KO - 1),
                        )
                        if ko == 7 and pend[0] is not None:
                            dstp, sop = pend[0]
                            pend[0] = None
                            rope_finish(dstp, sop, 512)
                    dst = kt_t[:, g2, s0 + sc * 512 : s0 + (sc + 1) * 512]
                    nc.scalar.copy(out=dst, in_=gk)
                    pend[0] = (dst, s0 + sc * 512)
            if pend[0] is not None:
                dstp, sop = pend[0]
                rope_finish(dstp, sop, 512)

        def dma_wq(hp):
            wq_hp = wq_pool.tile([P, KO, 256], _dt_bf16, tag="wq", name="wq_hp")
            for i in range(2):
                nc.sync.dma_start(
                    out=wq_hp[:, i * 16 : (i + 1) * 16, :],
                    in_=wqr[:, i * 16 : (i + 1) * 16, hp * 256 : (hp + 1) * 256],
                )
            return wq_hp

        def gen_q_proj(xt, hi, hp, qt_pair, wq_hp=None):
            """Q projection + rope for head pair hp; yields per subgroup.
            sc-outer so q[0:512] of both heads completes first; rope's perm
            matmul is deferred 8 ko into the next group to hide the scalar
            copy latency from the PE."""
            s0 = hi * SH
            if wq_hp is None:
                wq_hp = dma_wq(hp)
            pend = [None]

            def flush():
                if pend[0] is not None:
                    dst, so = pend[0]
                    pend[0] = None
                    rope_finish(dst, so, 512)

            for sc in range(2):
                for h2 in range(2):
                    gq = ps_mm.tile([P, 512], _dt_f32, tag="mm", name="gq")
                    for ko in range(KO):
                        nc.tensor.matmul(
                            gq,
                            lhsT=wq_hp[:, ko, h2 * P : (h2 + 1) * P],
                            rhs=xt[:, ko, sc * 512 : (sc + 1) * 512],
                            start=(ko == 0),
                            stop=(ko == KO - 1),
                        )
                        if ko == 7:
                            flush()
                        if ko % 4 == 3 and ko != KO - 1:
                            yield
                    dst = qt_pair[:, h2, sc * 512 : (sc + 1) * 512]
                    nc.scalar.copy(out=dst, in_=gq)
                    pend[0] = (dst, s0 + sc * 512)
                    yield
            flush()

        def gen_attn_pair(hp, hi, qt_pair):
            """Attention for head pair hp over q in [hi*1024, hi*1024+1024).

            q-block-outer (flash-style): for each 128-row q block, scores are
            computed TRANSPOSED (st[k, q] = K^T-block.T @ Q^T-block) for BOTH
            heads of the pair in one matmul (they share the kv head), exp'd on
            the scalar engine, then accumulated into one full-bank PSUM
            accumulator per head via A[q, dh|rowsum] += st_exp.T @ V_ext[kb].
            Only two accumulation groups are open at a time (bank rule)."""
            h0 = hp * 2
            g2 = h0 // 4
            for qbl in range(8):
                gqb = hi * 8 + qbl
                qsl = qt_pair[:, :, qbl * P : (qbl + 1) * P]  # [128, 2, 128]
                aA = ps_a.tile([P, 512], _dt_f32, tag="A", name="aA")
                aB = ps_a.tile([P, 512], _dt_f32, tag="A", name="aB")
                for kb0 in range(0, gqb + 1, 2):
                    n2 = min(2, gqb + 1 - kb0)
                    stp = ps_scr.tile([P, 512], _dt_f32, tag="scr", name="stp")
                    for j in range(n2):
                        kb = kb0 + j
                        nc.tensor.matmul(
                            stp[:, j * 256 : j * 256 + 256],
                            lhsT=kt_t[:, g2, kb * P : (kb + 1) * P],
                            rhs=qsl,
                            start=True,
                            stop=True,
                        )
                        if kb == gqb:
                            nc.vector.tensor_add(
                                stp[:, j * 256 : j * 256 + P],
                                stp[:, j * 256 : j * 256 + P],
                                maskT_t,
                            )
                            nc.vector.tensor_add(
                                stp[:, j * 256 + P : j * 256 + 256],
                                stp[:, j * 256 + P : j * 256 + 256],
                                maskT_t,
                            )
                    pbt = pb_pool.tile([P, 512], _dt_bf16, tag="pb", name="pbt")
                    nc.scalar.activation(
                        out=pbt[:, : n2 * 256],
                        in_=stp[:, : n2 * 256],
                        func=mybir.ActivationFunctionType.Exp,
                        scale=SCALE,
                    )
                    yield
                    for j in range(n2):
                        kb = kb0 + j
                        for h2, ap_ in ((0, aA), (1, aB)):
                            nc.tensor.matmul(
                                ap_[:, 0:129],
                                lhsT=pbt[:, j * 256 + h2 * P : j * 256 + h2 * P + P],
                                rhs=v_t[:, kb, g2, 0:129],
                                start=(kb == 0),
                                stop=(kb == gqb),
                            )
                for h2, ap_ in ((0, aA), (1, aB)):
                    h = h0 + h2
                    rinv = small.tile([P, 1], _dt_f32, tag="r", name="rinv")
                    nc.vector.reciprocal(rinv, ap_[:, 128:129])
                    nc.vector.tensor_scalar_mul(
                        a_t[:, gqb, h * P : (h + 1) * P],
                        ap_[:, 0:P],
                        rinv,
                    )
                yield

        # ================= main schedule =================
        # Startup: parallel DMA issue queues — x^T via the scalar queue,
        # weights via gpsimd, everything else on sync — so the V projection
        # can start after ~3MB instead of waiting out one serialized queue.
        wkv_v0 = dma_wkv_v()
        xt0 = xt_pool.tile([P, KO, SH], _dt_bf16, tag="xt", name="xt0")
        dma_xt(xt0, 0, nc.sync)
        wq0w = dma_wq(0)
        nc.sync.dma_start(out=cexp_t, in_=aps["cexp"])
        nc.sync.dma_start(out=sexp_t, in_=aps["sexp"])
        nc.sync.dma_start(out=perm_t, in_=aps["perm"])
        nc.sync.dma_start(out=ident_t, in_=aps["ident"])
        nc.sync.dma_start(out=maskT_t, in_=aps["maskT"])
        nc.sync.dma_start(
            out=v_t[:, :, :, DH : DH + 1],
            in_=aps["vones"].rearrange("p (a b c) -> p a b c", a=16, b=NKV_L),
        )
        _run(gen_v_proj(xt0, 0, wkv_v0))
        qt0 = qt_pool.tile([P, 2, SH], _dt_bf16, tag="qt", name="qt0")
        wq1w = dma_wq(1)
        _run(gen_q_proj(xt0, 0, 0, qt0, wq0w))
        k_proj(xt0, 0)
        qts = {0: qt0}
        wqs = {1: wq1w}
        for hp in range(3):
            if hp + 2 <= 3:
                wqs[hp + 2] = dma_wq(hp + 2)
            qts[hp + 1] = qt_pool.tile([P, 2, SH], _dt_bf16, tag="qt", name="qtn")
            _rr(
                (gen_attn_pair(hp, 0, qts[hp]), 2),
                (gen_q_proj(xt0, 0, hp + 1, qts[hp + 1], wqs[hp + 1]), 1),
            )
        # last pair of half 0 runs while half 1's x DMA + V projection proceed
        wkv_v1 = dma_wkv_v()
        xt1 = xt_pool.tile([P, KO, SH], _dt_bf16, tag="xt", name="xt1")
        dma_xt(xt1, SH, nc.sync)
        wq0bw = dma_wq(0)
        _rr((gen_attn_pair(3, 0, qts[3]), 2), (gen_v_proj(xt1, 1, wkv_v1), 1))

        # ---- half 1 ----
        qt0b = qt_pool.tile([P, 2, SH], _dt_bf16, tag="qt", name="qt0b")
        wq1bw = dma_wq(1)
        _run(gen_q_proj(xt1, 1, 0, qt0b, wq0bw))
        k_proj(xt1, 1)
        qts = {0: qt0b}
        wqs = {1: wq1bw}
        for hp in range(3):
            if hp + 2 <= 3:
                wqs[hp + 2] = dma_wq(hp + 2)
            qts[hp + 1] = qt_pool.tile([P, 2, SH], _dt_bf16, tag="qt", name="qtm")
            _rr(
                (gen_attn_pair(hp, 1, qts[hp]), 4),
                (gen_q_proj(xt1, 1, hp + 1, qts[hp + 1], wqs[hp + 1]), 1),
            )
        # wo weights stream into xt's (now dead) buffer while the last
        # attention pair runs; e-major halves so early (sb, ec) units unblock.
        wo_t = xt_pool.tile([P, NH_L, D], _dt_bf16, tag="xt", name="wo_t")
        for eh in range(2):
            nc.sync.dma_start(
                out=wo_t[:, :, eh * 2048 : (eh + 1) * 2048],
                in_=wor[:, :, eh * 2048 : (eh + 1) * 2048],
            )
        _run(gen_attn_pair(3, 1, qts[3]))
        c1.close()

        # ---- wo projection: out[s, e] = sum_hd A^T[hd, s-blk].T @ woT[hd, e] ----
        at_pool = g.enter_context(tc.tile_pool(name="atp", bufs=3))

        def at_build(sb):
            at = at_pool.tile([P, NH_L, P], _dt_bf16, tag="at", name="at")
            for hb4 in range(0, NH_L, 4):
                tp4 = ps_scr.tile([P, 4, P], _dt_bf16, tag="scr", name="tp4")
                for j in range(4):
                    nc.tensor.transpose(
                        tp4[:, j, :],
                        a_t[:, sb, (hb4 + j) * P : (hb4 + j + 1) * P],
                        ident_t,
                    )
                cp(at[:, hb4 : hb4 + 4, :], tp4)
            return at

        ps_wo = g.enter_context(tc.tile_pool(name="ps_wo", bufs=2, space="PSUM"))
        at = at_build(0)
        for sb in range(16):
            at_next = at_build(sb + 1) if sb + 1 < 16 else None
            for ec in range(8):
                pool = ps_mm if ec % 2 == 0 else ps_wo
                go = pool.tile([P, 512], _dt_f32, tag="mm", name="go")
                for hb in range(NH_L):
                    nc.tensor.matmul(
                        go,
                        lhsT=at[:, hb, :],
                        rhs=wo_t[:, hb, ec * 512 : (ec + 1) * 512],
                        start=(hb == 0),
                        stop=(hb == NH_L - 1),
                    )
                ost = ostage.tile([P, 512], _dt_f32, tag="ost", name="ost")
                cp(ost, go)
                nc.sync.dma_start(
                    out=out_ap[sb * P : (sb + 1) * P, ec * 512 : (ec + 1) * 512],
                    in_=ost,
                )
            at = at_next


def _split_excess_waits(nc, limit=1):
    """Walrus (pinned build) rejects instructions carrying more than ~2
    semaphore waits.  Hoist excess waits onto same-engine no-ops inserted
    immediately before the offending instruction: the engine executes the
    nop's waits first, so the AND-semantics of the wait set is preserved."""
    ctr = [0]
    for bb in nc.main_func.blocks:
        insts = list(bb.instructions)
        out = []
        changed = False
        for ins in insts:
            si = ins.sync_info
            waits = list(si.on_wait) if si and si.on_wait else []
            if len(waits) > limit:
                keep = waits[:limit]
                rest = waits[limit:]
                for i in range(0, len(rest), limit):
                    nop = mybir.InstNoOp(name=f"I-waitsplit-{ctr[0]}", ins=[], outs=[])
                    ctr[0] += 1
                    nop.engine = ins.engine
                    nop.sync_info = mybir.SyncInfo(
                        on_wait=rest[i : i + limit], on_update=[]
                    )
                    nc.register_instruction(nop)
                    out.append(nop)
                si.on_wait = keep
                changed = True
            out.append(ins)
        if changed:
            bb.instructions = out
    return ctr[0]


_PROGRAM_CACHE = {}


def build_program():
    if "nc" in _PROGRAM_CACHE:
        return _PROGRAM_CACHE["nc"]
    nc = bass.Bass("TRN2", target_bir_lowering=False, debug=False, num_devices=N_CORES)
    aps = {
        "xT": nc.dram_tensor("xT", [D, S], _dt_bf16, kind="ExternalInput").ap(),
        "wqT": nc.dram_tensor("wqT", [D, NH_L * DH], _dt_bf16, kind="ExternalInput").ap(),
        "wkvT": nc.dram_tensor("wkvT", [D, 512], _dt_bf16, kind="ExternalInput").ap(),
        "woT": nc.dram_tensor("woT", [NH_L * DH, D], _dt_bf16, kind="ExternalInput").ap(),
        "cexp": nc.dram_tensor("cexp", [P, S], _dt_bf16, kind="ExternalInput").ap(),
        "sexp": nc.dram_tensor("sexp", [P, S], _dt_bf16, kind="ExternalInput").ap(),
        "perm": nc.dram_tensor("perm", [P, P], _dt_bf16, kind="ExternalInput").ap(),
        "ident": nc.dram_tensor("ident", [P, P], _dt_bf16, kind="ExternalInput").ap(),
        "maskT": nc.dram_tensor("maskT", [P, P], _dt_f32, kind="ExternalInput").ap(),
        "vones": nc.dram_tensor("vones", [P, 32], _dt_bf16, kind="ExternalInput").ap(),
        "out": nc.dram_tensor("out", [S, D], _dt_f32, kind="ExternalOutput").ap(),
    }
    with tile.TileContext(nc) as tc:
        _emit(tc, aps)
    _split_excess_waits(nc, limit=1)
    _PROGRAM_CACHE["nc"] = nc
    return nc


def make_in_maps(x, freqs_cos, freqs_sin, mask, wq, wk, wv, wo):
    x = np.asarray(x, np.float32)
    freqs_cos = np.asarray(freqs_cos, np.float32)
    freqs_sin = np.asarray(freqs_sin, np.float32)
    mask = np.asarray(mask, np.float32)
    wq = np.asarray(wq, np.float32)
    wk = np.asarray(wk, np.float32)
    wv = np.asarray(wv, np.float32)
    wo = np.asarray(wo, np.float32)

    xb = [x[b].T.astype(BF16) for b in range(2)]  # (4096, 2048)
    cexp = np.repeat(freqs_cos.T, 2, axis=0).astype(BF16)  # (128, 2048)
    sx = np.repeat(freqs_sin.T, 2, axis=0).astype(np.float32)
    sx[0::2] *= -1.0
    sexp = sx.astype(BF16)
    perm = np.zeros((P, P), np.float32)
    idx = np.arange(P)
    perm[idx, idx ^ 1] = 1.0
    perm = perm.astype(BF16)
    ident = np.eye(P, dtype=np.float32).astype(BF16)
    maskT = np.ascontiguousarray(mask[:P, :P].T, dtype=np.float32)
    vones = np.ones((P, 32), BF16)

    in_maps = []
    for core in range(N_CORES):
        b, tp = core // 4, core % 4
        wqT = wq[tp * 1024 : (tp + 1) * 1024].T.astype(BF16)  # (4096, 1024)
        wkT = wk[tp * 256 : (tp + 1) * 256].T.astype(BF16)  # (4096, 256)
        wvT = wv[tp * 256 : (tp + 1) * 256].T.astype(BF16)
        wkvT = np.ascontiguousarray(np.concatenate([wkT, wvT], axis=1))
        woT = wo[:, tp * 1024 : (tp + 1) * 1024].T.astype(BF16)  # (1024, 4096)
        in_maps.append(
            {
                "xT": xb[b],
                "wqT": wqT,
                "wkvT": wkvT,
                "woT": woT,
                "cexp": cexp,
                "sexp": sexp,
                "perm": perm,
                "ident": ident,
                "maskT": maskT,
                "vones": vones,
            }
        )
    return in_maps


def run(inputs, trace=False):
    nc = build_program()
    in_maps = make_in_maps(
        inputs["x"],
        inputs["freqs_cos"],
        inputs["freqs_sin"],
        inputs["mask"],
        inputs["wq"],
        inputs["wk"],
        inputs["wv"],
        inputs["wo"],
    )
    res = run_bass_kernel_spmd(nc, in_maps, list(range(N_CORES)), trace=trace)
    out = np.zeros((2, S, D), np.float32)
    for core in range(N_CORES):
        out[core // 4] += np.asarray(res.results[core]["out"], np.float32)
    return out, res


def kernel(x, freqs_cos, freqs_sin, positions, mask, wq, wk, wv, wo):
    out, _ = run(
        {
            "x": x,
            "freqs_cos": freqs_cos,
            "freqs_sin": freqs_sin,
            "mask": mask,
            "wq": wq,
            "wk": wk,
            "wv": wv,
            "wo": wo,
        }
    )
    return out
